# revision 14
# baseline (speedup 1.0000x reference)
"""Trainium2 Bass kernel for causal self-attention with GQA + RoPE.

Problem: B=4, T=2048, C=2048, 16 q heads, 4 kv heads, head_dim=128.
Sharding: data-parallel over the 4 batches x tensor-parallel over the 2
kv-head groups -> 8 cores. Each core computes its batch's attention for
one group of 8 q heads / 2 kv heads plus the partial output projection;
the host sums the two partial output projections per batch.

Device layout notes:
 - x is passed transposed (xT[c, t]) so projection matmuls need no
   on-device transposes: qT[d, t] = Wq[c, d].T @ xT[c, t].
 - Scores are computed transposed, sT[kv, q] = kT.T_slice @ qT, so the
   P@V matmul (lhsT = v[kv, d], rhs = exp_sT[kv, q] -> yT[d, q]) and the
   output projection (lhsT = yT[d, t] chunks) need no transposes either.
 - Softmax skips the running max: scores are bounded (|s| < ~6 for this
   distribution), exp is safe in f32. Denominator = ones-vector matmul
   over exp_sT, reciprocal broadcast back over partitions via a K=1
   matmul.
 - RoPE uses a host-side permutation of the head dim into "half" layout
   (real parts in partitions 0-63, imag in 64-127) so rotate-half is two
   partition-contiguous SBUF->SBUF DMA copies. The k output is
   un-permuted on device by a matmul with a permutation matrix.
"""

import os
import sys

sys.path.insert(0, "/opt/trn_rl_repo")

import numpy as np
import ml_dtypes

import concourse.bass as bass
import concourse.mybir as mybir
import concourse.tile as tile
from concourse import bacc
from concourse.bass_utils import run_bass_kernel_spmd

BF16 = ml_dtypes.bfloat16
F32 = mybir.dt.float32
DT = mybir.dt.bfloat16

B, T, C = 4, 2048, 2048
H, HK, D = 16, 4, 128
P = 128
NCORES = 8
G = 2            # tensor-parallel groups
HL = H // G      # q heads per core (8)
HKL = HK // G    # kv heads per core (2)
DL = HL * D      # local q width (1024)
DKL = HKL * D    # local kv width (256)
KC = C // P      # 16 contraction chunks
NT = T // P      # 16 token tiles of 128
NJ = T // 512    # 4 q column tiles of 512
SCALE = 1.0 / float(np.sqrt(D))

_NC_CACHE = None


def _build_nc():
    nc = bacc.Bacc()

    xT = nc.declare_dram_parameter("xT", [C, T], DT, isOutput=False)
    wq = nc.declare_dram_parameter("wq", [C, DL], DT, isOutput=False)
    wk = nc.declare_dram_parameter("wk", [C, DKL], DT, isOutput=False)
    wv = nc.declare_dram_parameter("wv", [C, DKL], DT, isOutput=False)
    wc = nc.declare_dram_parameter("wc", [DL, C], DT, isOutput=False)
    cosb = nc.declare_dram_parameter("cosb", [P, T], DT, isOutput=False)
    sinb = nc.declare_dram_parameter("sinb", [P, T], DT, isOutput=False)
    unperm = nc.declare_dram_parameter("unperm", [P, P], DT, isOutput=False)
    tri = nc.declare_dram_parameter("tri", [P, P], DT, isOutput=False)

    y_out = nc.declare_dram_parameter("y", [T, C], F32, isOutput=True)
    k_out = nc.declare_dram_parameter("ko", [T, DKL], DT, isOutput=True)
    v_out = nc.declare_dram_parameter("vo", [T, DKL], DT, isOutput=True)

    with tile.TileContext(nc) as tc:
        with (
            tc.tile_pool(name="const", bufs=1) as cpool,
            tc.tile_pool(name="persist", bufs=1) as persist,
        ):
            cos_sb = cpool.tile([P, T], DT, tag="cos")
            sin_sb = cpool.tile([P, T], DT, tag="sin")
            up_sb = cpool.tile([P, P], DT, tag="unperm")
            tri_sb = cpool.tile([P, P], DT, tag="tri")
            ones_sb = cpool.tile([P, P], DT, tag="ones")
            nc.sync.dma_start(cos_sb[:], cosb[:])
            nc.sync.dma_start(sin_sb[:], sinb[:])
            nc.sync.dma_start(up_sb[:], unperm[:])
            nc.sync.dma_start(tri_sb[:], tri[:])
            nc.vector.memset(ones_sb[:], 1.0)

            qT_sb = persist.tile([P, HL, T], DT, tag="qT")
            kT_sb = persist.tile([P, HKL, T], DT, tag="kT")
            v_sb = persist.tile([P, NT, DKL], DT, tag="v")

            # ---------------- Phase A: projections + RoPE -------------
            with tc.tile_pool(name="proj", bufs=1) as proj:
                xT_sb = proj.tile([P, KC, T], DT, tag="xT")
                wq_sb = proj.tile([P, KC, DL], DT, tag="wq")
                wk_sb = proj.tile([P, KC, DKL], DT, tag="wk")
                wv_sb = proj.tile([P, KC, DKL], DT, tag="wv")
                # Split the loads per chunk so matmuls start after chunk 0.
                for kc in range(KC):
                    nc.sync.dma_start(wq_sb[:, kc], wq[kc * P:(kc + 1) * P, :])
                    nc.sync.dma_start(xT_sb[:, kc], xT[kc * P:(kc + 1) * P, :])
                    nc.sync.dma_start(wk_sb[:, kc], wk[kc * P:(kc + 1) * P, :])
                    nc.sync.dma_start(wv_sb[:, kc], wv[kc * P:(kc + 1) * P, :])

                with tc.tile_pool(name="psA", bufs=2, space="PSUM") as psA, \
                        nc.named_scope("projqk"):
                    # k and q projections (both get RoPE); k first so
                    # attention head 0 unblocks as early as possible.
                    targets = [("k", kh) for kh in range(HKL)] + [
                        ("q", h) for h in range(HL)
                    ]
                    for kind, idx in targets:
                        ps = [
                            psA.tile([P, 512], F32, tag=f"proj{j}", name=f"proj{j}")
                            for j in range(NJ)
                        ]
                        for kc in range(KC):
                            if kind == "q":
                                lhsT = wq_sb[:, kc, idx * D:(idx + 1) * D]
                            else:
                                lhsT = wk_sb[:, kc, idx * D:(idx + 1) * D]
                            for j in range(NJ):
                                nc.tensor.matmul(
                                    ps[j][:],
                                    lhsT,
                                    xT_sb[:, kc, j * 512:(j + 1) * 512],
                                    start=(kc == 0),
                                    stop=(kc == KC - 1),
                                )
                        stage = proj.tile([P, T], DT, tag="ropestage", bufs=2)
                        for j in range(NJ):
                            nc.scalar.copy(stage[:, j * 512:(j + 1) * 512], ps[j][:])
                        rot = proj.tile([P, T], DT, tag="rot", bufs=2)
                        nc.sync.dma_start(rot[0:64, :], stage[64:128, :])
                        nc.sync.dma_start(rot[64:128, :], stage[0:64, :])
                        dest = (
                            qT_sb[:, idx, :] if kind == "q" else kT_sb[:, idx, :]
                        )
                        nc.vector.tensor_mul(dest, stage[:], cos_sb[:])
                        tmp = proj.tile([P, T], DT, tag="ropetmp", bufs=2)
                        nc.vector.tensor_mul(tmp[:], rot[:], sin_sb[:])
                        nc.vector.tensor_add(dest, dest, tmp[:])

                with tc.tile_pool(name="psA2", bufs=2, space="PSUM") as psA2, \
                        nc.named_scope("vproj"):
                    # v projection: v[t, d] tiles (needs t on partitions)
                    for tt in range(NT):
                        vp = psA2.tile([P, DKL], F32, tag="vp", bufs=3)
                        for kc in range(KC):
                            nc.tensor.matmul(
                                vp[:],
                                xT_sb[:, kc, tt * P:(tt + 1) * P],
                                wv_sb[:, kc, :],
                                start=(kc == 0),
                                stop=(kc == KC - 1),
                            )
                        nc.scalar.copy(v_sb[:, tt, :], vp[:])
                    nc.sync.dma_start(
                        v_out.rearrange("(o p) d -> p o d", p=P), v_sb[:]
                    )

                    # k output: un-permute head dim back to interleaved
                    # layout via matmul with permutation matrix.
                    for kh in range(HKL):
                        kst = proj.tile([P, NT, P], DT, tag="kout", bufs=2)
                        for tt in range(NT):
                            kp = psA2.tile([P, P], F32, tag="ktr", bufs=3)
                            nc.tensor.matmul(
                                kp[:],
                                kT_sb[:, kh, tt * P:(tt + 1) * P],
                                up_sb[:],
                                start=True,
                                stop=True,
                            )
                            nc.scalar.copy(kst[:, tt, :], kp[:])
                        nc.sync.dma_start(
                            k_out.rearrange("(o p) d -> p o d", p=P)[
                                :, :, kh * D:(kh + 1) * D
                            ],
                            kst[:],
                        )

            # ---------------- Phases B+C: attention + out-proj --------
            with tc.tile_pool(name="post", bufs=1) as post:
                yT_sb = post.tile([P, HL, T], DT, tag="yT")
                # Hoist the Wc load so it overlaps the attention phase.
                wc_sb = post.tile([P, HL, C], DT, tag="wc")
                for kc in range(HL):
                    nc.sync.dma_start(wc_sb[:, kc], wc[kc * P:(kc + 1) * P, :])

                # Attention runs in two q-column passes of 1024; after each
                # pass the output projection for that half of the tokens is
                # emitted, so oproj matmuls fill PE gaps in the next pass.
                with (
                    tc.tile_pool(name="attn", bufs=1) as attn,
                    tc.tile_pool(name="psB", bufs=1, space="PSUM") as psB,
                    tc.tile_pool(name="psO", bufs=1, space="PSUM") as psO,
                ):
                    W = 1024  # q window per pass

                    def emit_oproj(pass_):
                        with nc.named_scope("oproj"):
                            for tt in range(8 * pass_, 8 * pass_ + 8):
                                ost = attn.tile([P, C], F32, tag="ost",
                                                bufs=2, name="ost")
                                for ncol in range(NJ):
                                    ops_ = psO.tile(
                                        [P, 512], F32, tag="o", bufs=2,
                                        name="ops")
                                    for hc in range(HL):
                                        nc.tensor.matmul(
                                            ops_[:],
                                            yT_sb[:, hc, tt * P:(tt + 1) * P],
                                            wc_sb[:, hc,
                                                  ncol * 512:(ncol + 1) * 512],
                                            start=(hc == 0),
                                            stop=(hc == HL - 1),
                                        )
                                    nc.vector.tensor_copy(
                                        ost[:, ncol * 512:(ncol + 1) * 512],
                                        ops_[:],
                                    )
                                nc.sync.dma_start(
                                    y_out[tt * P:(tt + 1) * P, :], ost[:]
                                )

                    with nc.named_scope("attn"):
                        for pass_ in range(2):
                            q0 = pass_ * W
                            nkt = (pass_ + 1) * 8
                            for h in range(HL):
                                kh = h // (HL // HKL)
                                yps = [
                                    psB.tile([P, 512], F32, tag=f"yps{jh}",
                                             bufs=1, name=f"yps{jh}")
                                    for jh in range(2)
                                ]
                                dps = [
                                    psB.tile([P, 512], F32, tag=f"dps{jh}",
                                             bufs=1, name=f"dps{jh}")
                                    for jh in range(2)
                                ]
                                # last kt contributing to each half-window
                                ktmax = [8 * pass_ + 4 * (jh + 1) - 1
                                         for jh in range(2)]
                                for kt in range(nkt):
                                    col0 = max(0, kt * P - q0)
                                    sps = psB.tile([P, W], F32, tag="s",
                                                   bufs=1, name="sps")
                                    for jh in range(2):
                                        c_lo = max(col0, jh * 512)
                                        c_hi = (jh + 1) * 512
                                        if c_lo >= c_hi:
                                            continue
                                        nc.tensor.matmul(
                                            sps[:, c_lo:c_hi],
                                            kT_sb[:, kh, kt * P:(kt + 1) * P],
                                            qT_sb[:, h, q0 + c_lo:q0 + c_hi],
                                            start=True,
                                            stop=True,
                                        )
                                    ex = attn.tile([P, W], DT, tag="exp",
                                                   bufs=3, name="ex")
                                    for jh in range(2):
                                        c_lo = max(col0, jh * 512)
                                        c_hi = (jh + 1) * 512
                                        if c_lo >= c_hi:
                                            continue
                                        nc.scalar.activation(
                                            ex[:, c_lo:c_hi],
                                            sps[:, c_lo:c_hi],
                                            mybir.ActivationFunctionType.Exp,
                                            scale=SCALE,
                                        )
                                    if kt * P >= q0:
                                        nc.vector.tensor_mul(
                                            ex[:, col0:col0 + P],
                                            ex[:, col0:col0 + P],
                                            tri_sb[:],
                                        )
                                    for jh in range(2):
                                        c_lo = max(col0, jh * 512)
                                        c_hi = (jh + 1) * 512
                                        if c_lo >= c_hi:
                                            continue
                                        nc.tensor.matmul(
                                            yps[jh][:, c_lo - jh * 512:],
                                            v_sb[:, kt, kh * D:(kh + 1) * D],
                                            ex[:, c_lo:c_hi],
                                            start=(kt == 0),
                                            stop=(kt == ktmax[jh]),
                                            skip_group_check=True,
                                        )
                                        # denominator, broadcast over all
                                        # partitions by an all-ones stationary
                                        nc.tensor.matmul(
                                            dps[jh][:, c_lo - jh * 512:],
                                            ones_sb[:],
                                            ex[:, c_lo:c_hi],
                                            start=(kt == 0),
                                            stop=(kt == ktmax[jh]),
                                            skip_group_check=True,
                                        )
                                for jh in range(2):
                                    rec = attn.tile([P, 512], F32, tag="rec",
                                                    bufs=2, name="rec")
                                    nc.vector.reciprocal(rec[:], dps[jh][:])
                                    nc.vector.tensor_mul(
                                        yT_sb[:, h,
                                              q0 + jh * 512:q0 + (jh + 1) * 512],
                                        yps[jh][:],
                                        rec[:],
                                    )
                            emit_oproj(pass_)

    nc.finalize()
    return nc


def _get_nc():
    global _NC_CACHE
    if _NC_CACHE is None:
        _NC_CACHE = _build_nc()
    return _NC_CACHE


def _host_prep(x, freqs_cos, freqs_sin, Wq, Wk, Wv, Wc):
    """Build the 8 per-core input maps (numpy, bf16)."""
    perm = np.empty(P, dtype=np.int64)
    perm[0:64] = np.arange(64) * 2        # real parts -> partitions 0..63
    perm[64:128] = np.arange(64) * 2 + 1  # imag parts -> partitions 64..127

    cosT = np.ascontiguousarray(freqs_cos.T)  # [64, T]
    sinT = np.ascontiguousarray(freqs_sin.T)
    cosb = np.concatenate([cosT, cosT], axis=0).astype(BF16)
    sinb = np.concatenate([-sinT, sinT], axis=0).astype(BF16)

    unperm = np.zeros((P, P), dtype=BF16)
    unperm[np.arange(P), perm] = 1.0  # M[d, perm[d]] = 1

    tri = (np.arange(P)[None, :] >= np.arange(P)[:, None]).astype(BF16)

    Wq_p = (
        Wq.reshape(C, H, D)[:, :, perm].reshape(C, H * D).astype(BF16)
    )
    Wk_p = (
        Wk.reshape(C, HK, D)[:, :, perm].reshape(C, HK * D).astype(BF16)
    )
    Wv_b = Wv.astype(BF16)
    Wc_b = Wc.astype(BF16)

    in_maps = []
    for core in range(NCORES):
        g = core // B
        b = core % B
        xT_c = np.ascontiguousarray(x[b].T).astype(BF16)
        in_maps.append(
            {
                "xT": xT_c,
                "wq": np.ascontiguousarray(Wq_p[:, g * DL:(g + 1) * DL]),
                "wk": np.ascontiguousarray(Wk_p[:, g * DKL:(g + 1) * DKL]),
                "wv": np.ascontiguousarray(Wv_b[:, g * DKL:(g + 1) * DKL]),
                "wc": np.ascontiguousarray(Wc_b[g * DL:(g + 1) * DL, :]),
                "cosb": cosb,
                "sinb": sinb,
                "unperm": unperm,
                "tri": tri,
            }
        )
    return in_maps


def kernel(x, freqs_cos, freqs_sin, Wq, Wk, Wv, Wc, _trace=False, _trace_kwargs=None):
    x = np.asarray(x, dtype=np.float32)
    freqs_cos = np.asarray(freqs_cos, dtype=np.float32)
    freqs_sin = np.asarray(freqs_sin, dtype=np.float32)
    Wq = np.asarray(Wq, dtype=np.float32)
    Wk = np.asarray(Wk, dtype=np.float32)
    Wv = np.asarray(Wv, dtype=np.float32)
    Wc = np.asarray(Wc, dtype=np.float32)

    in_maps = _host_prep(x, freqs_cos, freqs_sin, Wq, Wk, Wv, Wc)
    nc = _get_nc()
    res = run_bass_kernel_spmd(
        nc, in_maps, core_ids=list(range(NCORES)), trace=_trace,
        **(_trace_kwargs or {}),
    )
    results = res.results

    y = np.empty((B, T, C), dtype=np.float32)
    present_k = np.empty((B, T, HK, D), dtype=np.float32)
    present_v = np.empty((B, T, HK, D), dtype=np.float32)
    for b in range(B):
        y[b] = results[b]["y"] + results[B + b]["y"]
        for g in range(G):
            rb = results[g * B + b]
            ko = np.asarray(rb["ko"], dtype=np.float32)
            vo = np.asarray(rb["vo"], dtype=np.float32)
            for kh in range(HKL):
                present_k[b, :, g * HKL + kh, :] = ko[:, kh * D:(kh + 1) * D]
                present_v[b, :, g * HKL + kh, :] = vo[:, kh * D:(kh + 1) * D]

    kernel.last_exec_time_ns = res.exec_time_ns
    kernel.last_results = res
    return (y, present_k, present_v)


# revision 15
# speedup vs baseline: 1.4257x; 1.4257x over previous
"""Trainium2 Bass kernel for causal self-attention with GQA + RoPE.

Problem: B=4, T=2048, C=2048, 16 q heads, 4 kv heads, head_dim=128.
Sharding: data-parallel over the 4 batches x tensor-parallel over the 2
kv-head groups -> 8 cores. Each core computes its batch's attention for
one group of 8 q heads / 2 kv heads plus the partial output projection;
the host sums the two partial output projections per batch.

Device layout notes:
 - x is passed transposed (xT[c, t]) so projection matmuls need no
   on-device transposes: qT[d, t] = Wq[c, d].T @ xT[c, t].
 - Scores are computed transposed, sT[kv, q] = kT.T_slice @ qT, so the
   P@V matmul (lhsT = v[kv, d], rhs = exp_sT[kv, q] -> yT[d, q]) and the
   output projection (lhsT = yT[d, t] chunks) need no transposes either.
 - Softmax skips the running max: scores are bounded (|s| < ~6 for this
   distribution), exp is safe in f32. Denominator = ones-vector matmul
   over exp_sT, reciprocal broadcast back over partitions via a K=1
   matmul.
 - RoPE uses a host-side permutation of the head dim into "half" layout
   (real parts in partitions 0-63, imag in 64-127) so rotate-half is two
   partition-contiguous SBUF->SBUF DMA copies. The k output is
   un-permuted on device by a matmul with a permutation matrix.
"""

import os
import sys

sys.path.insert(0, "/opt/trn_rl_repo")

import numpy as np
import ml_dtypes

import concourse.bass as bass
import concourse.mybir as mybir
import concourse.tile as tile
from concourse import bacc
from concourse.bass_utils import run_bass_kernel_spmd

BF16 = ml_dtypes.bfloat16
F32 = mybir.dt.float32
DT = mybir.dt.bfloat16

B, T, C = 4, 2048, 2048
H, HK, D = 16, 4, 128
P = 128
NCORES = 8
G = 2            # tensor-parallel groups
HL = H // G      # q heads per core (8)
HKL = HK // G    # kv heads per core (2)
DL = HL * D      # local q width (1024)
DKL = HKL * D    # local kv width (256)
KC = C // P      # 16 contraction chunks
NT = T // P      # 16 token tiles of 128
NJ = T // 512    # 4 q column tiles of 512
SCALE = 1.0 / float(np.sqrt(D))

_NC_CACHE = None


def _build_nc():
    nc = bacc.Bacc()

    xT = nc.declare_dram_parameter("xT", [C, T], DT, isOutput=False)
    wq = nc.declare_dram_parameter("wq", [C, DL], DT, isOutput=False)
    wk = nc.declare_dram_parameter("wk", [C, DKL], DT, isOutput=False)
    wv = nc.declare_dram_parameter("wv", [C, DKL], DT, isOutput=False)
    wc = nc.declare_dram_parameter("wc", [DL, C], DT, isOutput=False)
    cosb = nc.declare_dram_parameter("cosb", [P, T], DT, isOutput=False)
    sinb = nc.declare_dram_parameter("sinb", [P, T], DT, isOutput=False)
    unperm = nc.declare_dram_parameter("unperm", [P, P], DT, isOutput=False)
    tri = nc.declare_dram_parameter("tri", [P, P], DT, isOutput=False)

    y_out = nc.declare_dram_parameter("y", [T, C], F32, isOutput=True)
    k_out = nc.declare_dram_parameter("ko", [T, DKL], DT, isOutput=True)
    v_out = nc.declare_dram_parameter("vo", [T, DKL], DT, isOutput=True)

    with tile.TileContext(nc) as tc:
        with (
            tc.tile_pool(name="const", bufs=1) as cpool,
            tc.tile_pool(name="persist", bufs=1) as persist,
        ):
            cos_sb = cpool.tile([P, T], DT, tag="cos")
            sin_sb = cpool.tile([P, T], DT, tag="sin")
            up_sb = cpool.tile([P, P], DT, tag="unperm")
            tri_sb = cpool.tile([P, P], DT, tag="tri")
            ones_sb = cpool.tile([P, P], DT, tag="ones")
            nc.sync.dma_start(cos_sb[:], cosb[:])
            nc.sync.dma_start(sin_sb[:], sinb[:])
            nc.sync.dma_start(up_sb[:], unperm[:])
            nc.sync.dma_start(tri_sb[:], tri[:])
            nc.vector.memset(ones_sb[:], 1.0)

            qT_sb = persist.tile([P, HL, T], DT, tag="qT")
            kT_sb = persist.tile([P, HKL, T], DT, tag="kT")
            v_sb = persist.tile([P, NT, DKL], DT, tag="v")

            # ---------------- Phase A: projections + RoPE -------------
            with tc.tile_pool(name="proj", bufs=1) as proj:
                xT_sb = proj.tile([P, KC, T], DT, tag="xT")
                wq_sb = proj.tile([P, KC, DL], DT, tag="wq")
                wk_sb = proj.tile([P, KC, DKL], DT, tag="wk")
                wv_sb = proj.tile([P, KC, DKL], DT, tag="wv")
                # Split the loads per chunk so matmuls start after chunk 0.
                for kc in range(KC):
                    nc.sync.dma_start(wq_sb[:, kc], wq[kc * P:(kc + 1) * P, :])
                    nc.sync.dma_start(xT_sb[:, kc], xT[kc * P:(kc + 1) * P, :])
                    nc.sync.dma_start(wk_sb[:, kc], wk[kc * P:(kc + 1) * P, :])
                    nc.sync.dma_start(wv_sb[:, kc], wv[kc * P:(kc + 1) * P, :])

                with tc.tile_pool(name="psA", bufs=2, space="PSUM") as psA, \
                        nc.named_scope("projqk"):
                    # k and q projections (both get RoPE); k first so
                    # attention head 0 unblocks as early as possible.
                    targets = [("k", kh) for kh in range(HKL)] + [
                        ("q", h) for h in range(HL)
                    ]
                    for kind, idx in targets:
                        ps = [
                            psA.tile([P, 512], F32, tag=f"proj{j}", name=f"proj{j}")
                            for j in range(NJ)
                        ]
                        for kc in range(KC):
                            if kind == "q":
                                lhsT = wq_sb[:, kc, idx * D:(idx + 1) * D]
                            else:
                                lhsT = wk_sb[:, kc, idx * D:(idx + 1) * D]
                            for j in range(NJ):
                                nc.tensor.matmul(
                                    ps[j][:],
                                    lhsT,
                                    xT_sb[:, kc, j * 512:(j + 1) * 512],
                                    start=(kc == 0),
                                    stop=(kc == KC - 1),
                                )
                        stage = proj.tile([P, T], DT, tag="ropestage", bufs=2)
                        for j in range(NJ):
                            nc.scalar.copy(stage[:, j * 512:(j + 1) * 512], ps[j][:])
                        rot = proj.tile([P, T], DT, tag="rot", bufs=2)
                        nc.sync.dma_start(rot[0:64, :], stage[64:128, :])
                        nc.sync.dma_start(rot[64:128, :], stage[0:64, :])
                        dest = (
                            qT_sb[:, idx, :] if kind == "q" else kT_sb[:, idx, :]
                        )
                        nc.vector.tensor_mul(dest, stage[:], cos_sb[:])
                        tmp = proj.tile([P, T], DT, tag="ropetmp", bufs=2)
                        nc.vector.tensor_mul(tmp[:], rot[:], sin_sb[:])
                        nc.vector.tensor_add(dest, dest, tmp[:])

                with tc.tile_pool(name="psA2", bufs=2, space="PSUM") as psA2, \
                        nc.named_scope("vproj"):
                    # v projection: v[t, d] tiles (needs t on partitions)
                    for tt in range(NT):
                        vp = psA2.tile([P, DKL], F32, tag="vp", bufs=3)
                        for kc in range(KC):
                            nc.tensor.matmul(
                                vp[:],
                                xT_sb[:, kc, tt * P:(tt + 1) * P],
                                wv_sb[:, kc, :],
                                start=(kc == 0),
                                stop=(kc == KC - 1),
                            )
                        nc.scalar.copy(v_sb[:, tt, :], vp[:])
                    nc.sync.dma_start(
                        v_out.rearrange("(o p) d -> p o d", p=P), v_sb[:]
                    )

                    # k output: un-permute head dim back to interleaved
                    # layout via matmul with permutation matrix.
                    for kh in range(HKL):
                        kst = proj.tile([P, NT, P], DT, tag="kout", bufs=2)
                        for tt in range(NT):
                            kp = psA2.tile([P, P], F32, tag="ktr", bufs=3)
                            nc.tensor.matmul(
                                kp[:],
                                kT_sb[:, kh, tt * P:(tt + 1) * P],
                                up_sb[:],
                                start=True,
                                stop=True,
                            )
                            nc.scalar.copy(kst[:, tt, :], kp[:])
                        nc.sync.dma_start(
                            k_out.rearrange("(o p) d -> p o d", p=P)[
                                :, :, kh * D:(kh + 1) * D
                            ],
                            kst[:],
                        )

            # ---------------- Phases B+C: attention + out-proj --------
            with tc.tile_pool(name="post", bufs=1) as post:
                yT_sb = post.tile([P, HL, T], DT, tag="yT")
                # Hoist the Wc load so it overlaps the attention phase.
                wc_sb = post.tile([P, HL, C], DT, tag="wc")
                for kc in range(HL):
                    nc.sync.dma_start(wc_sb[:, kc], wc[kc * P:(kc + 1) * P, :])

                with (
                    tc.tile_pool(name="attn", bufs=1) as attn,
                    tc.tile_pool(name="psB", bufs=1, space="PSUM") as psB,
                    nc.named_scope("attn"),
                ):
                    for h in range(HL):
                        kh = h // (HL // HKL)
                        for j in range(NJ):
                            yps = psB.tile([P, 512], F32, tag="yT", bufs=3)
                            dps = psB.tile([P, 512], F32, tag="den", bufs=3)
                            nkt = 4 * j + 4
                            for kt in range(nkt):
                                col0 = max(0, kt * P - j * 512)
                                n = 512 - col0
                                sps = psB.tile([P, 512], F32, tag="s", bufs=2)
                                nc.tensor.matmul(
                                    sps[:, :n],
                                    kT_sb[:, kh, kt * P:(kt + 1) * P],
                                    qT_sb[:, h, j * 512 + col0:(j + 1) * 512],
                                    start=True,
                                    stop=True,
                                )
                                ex = attn.tile([P, 512], DT, tag="exp", bufs=6)
                                nc.scalar.activation(
                                    ex[:, :n],
                                    sps[:, :n],
                                    mybir.ActivationFunctionType.Exp,
                                    scale=SCALE,
                                )
                                if kt >= 4 * j:
                                    nc.vector.tensor_mul(
                                        ex[:, 0:P], ex[:, 0:P], tri_sb[:]
                                    )
                                nc.tensor.matmul(
                                    yps[:, col0:],
                                    v_sb[:, kt, kh * D:(kh + 1) * D],
                                    ex[:, :n],
                                    start=(kt == 0),
                                    stop=(kt == nkt - 1),
                                    skip_group_check=True,
                                )
                                # denominator, broadcast over all partitions
                                # by an all-ones stationary
                                nc.tensor.matmul(
                                    dps[:, col0:],
                                    ones_sb[:],
                                    ex[:, :n],
                                    start=(kt == 0),
                                    stop=(kt == nkt - 1),
                                    skip_group_check=True,
                                )
                            rec = attn.tile([P, 512], F32, tag="rec", bufs=3)
                            nc.vector.reciprocal(rec[:], dps[:])
                            nc.vector.tensor_mul(
                                yT_sb[:, h, j * 512:(j + 1) * 512],
                                yps[:],
                                rec[:],
                            )

                # out projection: y[t, :] = sum_h yT_h.T @ Wc rows
                with (
                    tc.tile_pool(name="oproj", bufs=1) as op,
                    tc.tile_pool(name="psC", bufs=2, space="PSUM") as psC,
                    nc.named_scope("oproj"),
                ):
                    for tt in range(NT):
                        ost = op.tile([P, C], F32, tag="ost", bufs=3)
                        for ncol in range(NJ):
                            ops_ = psC.tile([P, 512], F32, tag="o", bufs=4)
                            for hc in range(HL):
                                nc.tensor.matmul(
                                    ops_[:],
                                    yT_sb[:, hc, tt * P:(tt + 1) * P],
                                    wc_sb[:, hc, ncol * 512:(ncol + 1) * 512],
                                    start=(hc == 0),
                                    stop=(hc == HL - 1),
                                )
                            nc.vector.tensor_copy(
                                ost[:, ncol * 512:(ncol + 1) * 512], ops_[:]
                            )
                        nc.sync.dma_start(y_out[tt * P:(tt + 1) * P, :], ost[:])

    nc.finalize()
    return nc


def _get_nc():
    global _NC_CACHE
    if _NC_CACHE is None:
        _NC_CACHE = _build_nc()
    return _NC_CACHE


def _host_prep(x, freqs_cos, freqs_sin, Wq, Wk, Wv, Wc):
    """Build the 8 per-core input maps (numpy, bf16)."""
    perm = np.empty(P, dtype=np.int64)
    perm[0:64] = np.arange(64) * 2        # real parts -> partitions 0..63
    perm[64:128] = np.arange(64) * 2 + 1  # imag parts -> partitions 64..127

    cosT = np.ascontiguousarray(freqs_cos.T)  # [64, T]
    sinT = np.ascontiguousarray(freqs_sin.T)
    cosb = np.concatenate([cosT, cosT], axis=0).astype(BF16)
    sinb = np.concatenate([-sinT, sinT], axis=0).astype(BF16)

    unperm = np.zeros((P, P), dtype=BF16)
    unperm[np.arange(P), perm] = 1.0  # M[d, perm[d]] = 1

    tri = (np.arange(P)[None, :] >= np.arange(P)[:, None]).astype(BF16)

    Wq_p = (
        Wq.reshape(C, H, D)[:, :, perm].reshape(C, H * D).astype(BF16)
    )
    Wk_p = (
        Wk.reshape(C, HK, D)[:, :, perm].reshape(C, HK * D).astype(BF16)
    )
    Wv_b = Wv.astype(BF16)
    Wc_b = Wc.astype(BF16)

    in_maps = []
    for core in range(NCORES):
        g = core // B
        b = core % B
        xT_c = np.ascontiguousarray(x[b].T).astype(BF16)
        in_maps.append(
            {
                "xT": xT_c,
                "wq": np.ascontiguousarray(Wq_p[:, g * DL:(g + 1) * DL]),
                "wk": np.ascontiguousarray(Wk_p[:, g * DKL:(g + 1) * DKL]),
                "wv": np.ascontiguousarray(Wv_b[:, g * DKL:(g + 1) * DKL]),
                "wc": np.ascontiguousarray(Wc_b[g * DL:(g + 1) * DL, :]),
                "cosb": cosb,
                "sinb": sinb,
                "unperm": unperm,
                "tri": tri,
            }
        )
    return in_maps


def kernel(x, freqs_cos, freqs_sin, Wq, Wk, Wv, Wc, _trace=False, _trace_kwargs=None):
    x = np.asarray(x, dtype=np.float32)
    freqs_cos = np.asarray(freqs_cos, dtype=np.float32)
    freqs_sin = np.asarray(freqs_sin, dtype=np.float32)
    Wq = np.asarray(Wq, dtype=np.float32)
    Wk = np.asarray(Wk, dtype=np.float32)
    Wv = np.asarray(Wv, dtype=np.float32)
    Wc = np.asarray(Wc, dtype=np.float32)

    in_maps = _host_prep(x, freqs_cos, freqs_sin, Wq, Wk, Wv, Wc)
    nc = _get_nc()
    res = run_bass_kernel_spmd(
        nc, in_maps, core_ids=list(range(NCORES)), trace=_trace,
        **(_trace_kwargs or {}),
    )
    results = res.results

    y = np.empty((B, T, C), dtype=np.float32)
    present_k = np.empty((B, T, HK, D), dtype=np.float32)
    present_v = np.empty((B, T, HK, D), dtype=np.float32)
    for b in range(B):
        y[b] = results[b]["y"] + results[B + b]["y"]
        for g in range(G):
            rb = results[g * B + b]
            ko = np.asarray(rb["ko"], dtype=np.float32)
            vo = np.asarray(rb["vo"], dtype=np.float32)
            for kh in range(HKL):
                present_k[b, :, g * HKL + kh, :] = ko[:, kh * D:(kh + 1) * D]
                present_v[b, :, g * HKL + kh, :] = vo[:, kh * D:(kh + 1) * D]

    kernel.last_exec_time_ns = res.exec_time_ns
    kernel.last_results = res
    return (y, present_k, present_v)


# revision 16
# speedup vs baseline: 1.4267x; 1.0007x over previous
"""Trainium2 Bass kernel for causal self-attention with GQA + RoPE.

Problem: B=4, T=2048, C=2048, 16 q heads, 4 kv heads, head_dim=128.
Sharding: data-parallel over the 4 batches x tensor-parallel over the 2
kv-head groups -> 8 cores. Each core computes its batch's attention for
one group of 8 q heads / 2 kv heads plus the partial output projection;
the host sums the two partial output projections per batch.

Device layout notes:
 - x is passed transposed (xT[c, t]) so projection matmuls need no
   on-device transposes: qT[d, t] = Wq[c, d].T @ xT[c, t].
 - Scores are computed transposed, sT[kv, q] = kT.T_slice @ qT, so the
   P@V matmul (lhsT = v[kv, d], rhs = exp_sT[kv, q] -> yT[d, q]) and the
   output projection (lhsT = yT[d, t] chunks) need no transposes either.
 - Softmax skips the running max: scores are bounded (|s| < ~6 for this
   distribution), exp is safe in f32. Denominator = ones-vector matmul
   over exp_sT, reciprocal broadcast back over partitions via a K=1
   matmul.
 - RoPE uses a host-side permutation of the head dim into "half" layout
   (real parts in partitions 0-63, imag in 64-127) so rotate-half is two
   partition-contiguous SBUF->SBUF DMA copies. The k output is
   un-permuted on device by a matmul with a permutation matrix.
"""

import os
import sys

sys.path.insert(0, "/opt/trn_rl_repo")

import numpy as np
import ml_dtypes

import concourse.bass as bass
import concourse.mybir as mybir
import concourse.tile as tile
from concourse import bacc
from concourse.bass_utils import run_bass_kernel_spmd

BF16 = ml_dtypes.bfloat16
F32 = mybir.dt.float32
DT = mybir.dt.bfloat16

B, T, C = 4, 2048, 2048
H, HK, D = 16, 4, 128
P = 128
NCORES = 8
G = 2            # tensor-parallel groups
HL = H // G      # q heads per core (8)
HKL = HK // G    # kv heads per core (2)
DL = HL * D      # local q width (1024)
DKL = HKL * D    # local kv width (256)
KC = C // P      # 16 contraction chunks
NT = T // P      # 16 token tiles of 128
NJ = T // 512    # 4 q column tiles of 512
SCALE = 1.0 / float(np.sqrt(D))

_NC_CACHE = None


def _build_nc():
    nc = bacc.Bacc()

    xT = nc.declare_dram_parameter("xT", [C, T], DT, isOutput=False)
    wq = nc.declare_dram_parameter("wq", [C, DL], DT, isOutput=False)
    wk = nc.declare_dram_parameter("wk", [C, DKL], DT, isOutput=False)
    wv = nc.declare_dram_parameter("wv", [C, DKL], DT, isOutput=False)
    wc = nc.declare_dram_parameter("wc", [DL, C], DT, isOutput=False)
    cosb = nc.declare_dram_parameter("cosb", [P, T], DT, isOutput=False)
    sinb = nc.declare_dram_parameter("sinb", [P, T], DT, isOutput=False)
    unperm = nc.declare_dram_parameter("unperm", [P, P], DT, isOutput=False)
    tri = nc.declare_dram_parameter("tri", [P, P], DT, isOutput=False)

    y_out = nc.declare_dram_parameter("y", [T, C], F32, isOutput=True)
    k_out = nc.declare_dram_parameter("ko", [T, DKL], DT, isOutput=True)
    v_out = nc.declare_dram_parameter("vo", [T, DKL], DT, isOutput=True)

    with tile.TileContext(nc) as tc:
        with (
            tc.tile_pool(name="const", bufs=1) as cpool,
            tc.tile_pool(name="persist", bufs=1) as persist,
        ):
            cos_sb = cpool.tile([P, T], DT, tag="cos")
            sin_sb = cpool.tile([P, T], DT, tag="sin")
            up_sb = cpool.tile([P, P], DT, tag="unperm")
            tri_sb = cpool.tile([P, P], DT, tag="tri")
            ones_sb = cpool.tile([P, P], DT, tag="ones")
            nc.sync.dma_start(cos_sb[:], cosb[:])
            nc.sync.dma_start(sin_sb[:], sinb[:])
            nc.sync.dma_start(up_sb[:], unperm[:])
            nc.sync.dma_start(tri_sb[:], tri[:])
            nc.vector.memset(ones_sb[:], 1.0)

            qT_sb = persist.tile([P, HL, T], DT, tag="qT")
            kT_sb = persist.tile([P, HKL, T], DT, tag="kT")
            v_sb = persist.tile([P, NT, DKL], DT, tag="v")

            # ---------------- Phase A: projections + RoPE -------------
            with tc.tile_pool(name="proj", bufs=1) as proj:
                xT_sb = proj.tile([P, KC, T], DT, tag="xT")
                wq_sb = proj.tile([P, KC, DL], DT, tag="wq")
                wk_sb = proj.tile([P, KC, DKL], DT, tag="wk")
                wv_sb = proj.tile([P, KC, DKL], DT, tag="wv")
                # Split the loads per chunk so matmuls start after chunk 0.
                for kc in range(KC):
                    nc.sync.dma_start(wq_sb[:, kc], wq[kc * P:(kc + 1) * P, :])
                    nc.sync.dma_start(xT_sb[:, kc], xT[kc * P:(kc + 1) * P, :])
                    nc.sync.dma_start(wk_sb[:, kc], wk[kc * P:(kc + 1) * P, :])
                    nc.sync.dma_start(wv_sb[:, kc], wv[kc * P:(kc + 1) * P, :])

                with tc.tile_pool(name="psA", bufs=2, space="PSUM") as psA, \
                        nc.named_scope("projqk"):
                    # k and q projections (both get RoPE); k first so
                    # attention head 0 unblocks as early as possible.
                    targets = [("k", kh) for kh in range(HKL)] + [
                        ("q", h) for h in range(HL)
                    ]
                    for kind, idx in targets:
                        ps = [
                            psA.tile([P, 512], F32, tag=f"proj{j}", name=f"proj{j}")
                            for j in range(NJ)
                        ]
                        for kc in range(KC):
                            if kind == "q":
                                lhsT = wq_sb[:, kc, idx * D:(idx + 1) * D]
                            else:
                                lhsT = wk_sb[:, kc, idx * D:(idx + 1) * D]
                            for j in range(NJ):
                                nc.tensor.matmul(
                                    ps[j][:],
                                    lhsT,
                                    xT_sb[:, kc, j * 512:(j + 1) * 512],
                                    start=(kc == 0),
                                    stop=(kc == KC - 1),
                                )
                        stage = proj.tile([P, T], DT, tag="ropestage", bufs=2)
                        for j in range(NJ):
                            nc.scalar.copy(stage[:, j * 512:(j + 1) * 512], ps[j][:])
                        rot = proj.tile([P, T], DT, tag="rot", bufs=2)
                        nc.sync.dma_start(rot[0:64, :], stage[64:128, :])
                        nc.sync.dma_start(rot[64:128, :], stage[0:64, :])
                        dest = (
                            qT_sb[:, idx, :] if kind == "q" else kT_sb[:, idx, :]
                        )
                        nc.vector.tensor_mul(dest, stage[:], cos_sb[:])
                        tmp = proj.tile([P, T], DT, tag="ropetmp", bufs=2)
                        nc.vector.tensor_mul(tmp[:], rot[:], sin_sb[:])
                        nc.vector.tensor_add(dest, dest, tmp[:])

                with tc.tile_pool(name="psA2", bufs=2, space="PSUM") as psA2, \
                        nc.named_scope("vproj"):
                    # v projection: v[t, d] tiles (needs t on partitions)
                    for tt in range(NT):
                        vp = psA2.tile([P, DKL], F32, tag="vp", bufs=3)
                        for kc in range(KC):
                            nc.tensor.matmul(
                                vp[:],
                                xT_sb[:, kc, tt * P:(tt + 1) * P],
                                wv_sb[:, kc, :],
                                start=(kc == 0),
                                stop=(kc == KC - 1),
                            )
                        nc.scalar.copy(v_sb[:, tt, :], vp[:])
                    nc.sync.dma_start(
                        v_out.rearrange("(o p) d -> p o d", p=P), v_sb[:]
                    )

                    # k output: un-permute head dim back to interleaved
                    # layout via matmul with permutation matrix.
                    for kh in range(HKL):
                        kst = proj.tile([P, NT, P], DT, tag="kout", bufs=2)
                        for tt in range(NT):
                            kp = psA2.tile([P, P], F32, tag="ktr", bufs=3)
                            nc.tensor.matmul(
                                kp[:],
                                kT_sb[:, kh, tt * P:(tt + 1) * P],
                                up_sb[:],
                                start=True,
                                stop=True,
                            )
                            nc.scalar.copy(kst[:, tt, :], kp[:])
                        nc.sync.dma_start(
                            k_out.rearrange("(o p) d -> p o d", p=P)[
                                :, :, kh * D:(kh + 1) * D
                            ],
                            kst[:],
                        )

            # ---------------- Phases B+C: attention + out-proj --------
            with tc.tile_pool(name="post", bufs=1) as post:
                yT_sb = post.tile([P, HL, T], DT, tag="yT")
                # Hoist the Wc load so it overlaps the attention phase.
                wc_sb = post.tile([P, HL, C], DT, tag="wc")
                for kc in range(HL):
                    nc.sync.dma_start(wc_sb[:, kc], wc[kc * P:(kc + 1) * P, :])

                with (
                    tc.tile_pool(name="attn", bufs=1) as attn,
                    tc.tile_pool(name="psB", bufs=1, space="PSUM") as psB,
                    nc.named_scope("attn"),
                ):
                    for hp in range(HL // 2):
                      for j in range(NJ):
                        for h in (2 * hp, 2 * hp + 1):
                            kh = h // (HL // HKL)
                            yps = psB.tile([P, 512], F32, tag="yT", bufs=3)
                            dps = psB.tile([P, 512], F32, tag="den", bufs=3)
                            nkt = 4 * j + 4
                            for kt in range(nkt):
                                col0 = max(0, kt * P - j * 512)
                                n = 512 - col0
                                sps = psB.tile([P, 512], F32, tag="s", bufs=2)
                                nc.tensor.matmul(
                                    sps[:, :n],
                                    kT_sb[:, kh, kt * P:(kt + 1) * P],
                                    qT_sb[:, h, j * 512 + col0:(j + 1) * 512],
                                    start=True,
                                    stop=True,
                                )
                                ex = attn.tile([P, 512], DT, tag="exp", bufs=6)
                                nc.scalar.activation(
                                    ex[:, :n],
                                    sps[:, :n],
                                    mybir.ActivationFunctionType.Exp,
                                    scale=SCALE,
                                )
                                if kt >= 4 * j:
                                    nc.vector.tensor_mul(
                                        ex[:, 0:P], ex[:, 0:P], tri_sb[:]
                                    )
                                nc.tensor.matmul(
                                    yps[:, col0:],
                                    v_sb[:, kt, kh * D:(kh + 1) * D],
                                    ex[:, :n],
                                    start=(kt == 0),
                                    stop=(kt == nkt - 1),
                                    skip_group_check=True,
                                )
                                # denominator, broadcast over all partitions
                                # by an all-ones stationary
                                nc.tensor.matmul(
                                    dps[:, col0:],
                                    ones_sb[:],
                                    ex[:, :n],
                                    start=(kt == 0),
                                    stop=(kt == nkt - 1),
                                    skip_group_check=True,
                                )
                            rec = attn.tile([P, 512], F32, tag="rec", bufs=3)
                            nc.vector.reciprocal(rec[:], dps[:])
                            nc.vector.tensor_mul(
                                yT_sb[:, h, j * 512:(j + 1) * 512],
                                yps[:],
                                rec[:],
                            )

                # out projection: y[t, :] = sum_h yT_h.T @ Wc rows
                with (
                    tc.tile_pool(name="oproj", bufs=1) as op,
                    tc.tile_pool(name="psC", bufs=2, space="PSUM") as psC,
                    nc.named_scope("oproj"),
                ):
                    for tt in range(NT):
                        ost = op.tile([P, C], F32, tag="ost", bufs=3)
                        for ncol in range(NJ):
                            ops_ = psC.tile([P, 512], F32, tag="o", bufs=4)
                            for hc in range(HL):
                                nc.tensor.matmul(
                                    ops_[:],
                                    yT_sb[:, hc, tt * P:(tt + 1) * P],
                                    wc_sb[:, hc, ncol * 512:(ncol + 1) * 512],
                                    start=(hc == 0),
                                    stop=(hc == HL - 1),
                                )
                            nc.vector.tensor_copy(
                                ost[:, ncol * 512:(ncol + 1) * 512], ops_[:]
                            )
                        nc.sync.dma_start(y_out[tt * P:(tt + 1) * P, :], ost[:])

    nc.finalize()
    return nc


def _get_nc():
    global _NC_CACHE
    if _NC_CACHE is None:
        _NC_CACHE = _build_nc()
    return _NC_CACHE


def _host_prep(x, freqs_cos, freqs_sin, Wq, Wk, Wv, Wc):
    """Build the 8 per-core input maps (numpy, bf16)."""
    perm = np.empty(P, dtype=np.int64)
    perm[0:64] = np.arange(64) * 2        # real parts -> partitions 0..63
    perm[64:128] = np.arange(64) * 2 + 1  # imag parts -> partitions 64..127

    cosT = np.ascontiguousarray(freqs_cos.T)  # [64, T]
    sinT = np.ascontiguousarray(freqs_sin.T)
    cosb = np.concatenate([cosT, cosT], axis=0).astype(BF16)
    sinb = np.concatenate([-sinT, sinT], axis=0).astype(BF16)

    unperm = np.zeros((P, P), dtype=BF16)
    unperm[np.arange(P), perm] = 1.0  # M[d, perm[d]] = 1

    tri = (np.arange(P)[None, :] >= np.arange(P)[:, None]).astype(BF16)

    Wq_p = (
        Wq.reshape(C, H, D)[:, :, perm].reshape(C, H * D).astype(BF16)
    )
    Wk_p = (
        Wk.reshape(C, HK, D)[:, :, perm].reshape(C, HK * D).astype(BF16)
    )
    Wv_b = Wv.astype(BF16)
    Wc_b = Wc.astype(BF16)

    in_maps = []
    for core in range(NCORES):
        g = core // B
        b = core % B
        xT_c = np.ascontiguousarray(x[b].T).astype(BF16)
        in_maps.append(
            {
                "xT": xT_c,
                "wq": np.ascontiguousarray(Wq_p[:, g * DL:(g + 1) * DL]),
                "wk": np.ascontiguousarray(Wk_p[:, g * DKL:(g + 1) * DKL]),
                "wv": np.ascontiguousarray(Wv_b[:, g * DKL:(g + 1) * DKL]),
                "wc": np.ascontiguousarray(Wc_b[g * DL:(g + 1) * DL, :]),
                "cosb": cosb,
                "sinb": sinb,
                "unperm": unperm,
                "tri": tri,
            }
        )
    return in_maps


def kernel(x, freqs_cos, freqs_sin, Wq, Wk, Wv, Wc, _trace=False, _trace_kwargs=None):
    x = np.asarray(x, dtype=np.float32)
    freqs_cos = np.asarray(freqs_cos, dtype=np.float32)
    freqs_sin = np.asarray(freqs_sin, dtype=np.float32)
    Wq = np.asarray(Wq, dtype=np.float32)
    Wk = np.asarray(Wk, dtype=np.float32)
    Wv = np.asarray(Wv, dtype=np.float32)
    Wc = np.asarray(Wc, dtype=np.float32)

    in_maps = _host_prep(x, freqs_cos, freqs_sin, Wq, Wk, Wv, Wc)
    nc = _get_nc()
    res = run_bass_kernel_spmd(
        nc, in_maps, core_ids=list(range(NCORES)), trace=_trace,
        **(_trace_kwargs or {}),
    )
    results = res.results

    y = np.empty((B, T, C), dtype=np.float32)
    present_k = np.empty((B, T, HK, D), dtype=np.float32)
    present_v = np.empty((B, T, HK, D), dtype=np.float32)
    for b in range(B):
        y[b] = results[b]["y"] + results[B + b]["y"]
        for g in range(G):
            rb = results[g * B + b]
            ko = np.asarray(rb["ko"], dtype=np.float32)
            vo = np.asarray(rb["vo"], dtype=np.float32)
            for kh in range(HKL):
                present_k[b, :, g * HKL + kh, :] = ko[:, kh * D:(kh + 1) * D]
                present_v[b, :, g * HKL + kh, :] = vo[:, kh * D:(kh + 1) * D]

    kernel.last_exec_time_ns = res.exec_time_ns
    kernel.last_results = res
    return (y, present_k, present_v)


# revision 21
# speedup vs baseline: 1.4715x; 1.0314x over previous
"""Trainium2 Bass kernel for causal self-attention with GQA + RoPE.

Problem: B=4, T=2048, C=2048, 16 q heads, 4 kv heads, head_dim=128.
Sharding: data-parallel over the 4 batches x tensor-parallel over the 2
kv-head groups -> 8 cores. Each core computes its batch's attention for
one group of 8 q heads / 2 kv heads plus the partial output projection;
the host sums the two partial output projections per batch.

Device layout notes:
 - x is passed transposed (xT[c, t]) so projection matmuls need no
   on-device transposes: qT[d, t] = Wq[c, d].T @ xT[c, t].
 - Scores are computed transposed, sT[kv, q] = kT.T_slice @ qT, so the
   P@V matmul (lhsT = v[kv, d], rhs = exp_sT[kv, q] -> yT[d, q]) and the
   output projection (lhsT = yT[d, t] chunks) need no transposes either.
 - Softmax skips the running max: scores are bounded (|s| < ~6 for this
   distribution), exp is safe in f32. Denominator = ones-vector matmul
   over exp_sT, reciprocal broadcast back over partitions via a K=1
   matmul.
 - RoPE uses a host-side permutation of the head dim into "half" layout
   (real parts in partitions 0-63, imag in 64-127) so rotate-half is two
   partition-contiguous SBUF->SBUF DMA copies. The k output is
   un-permuted on device by a matmul with a permutation matrix.
"""

import os
import sys

sys.path.insert(0, "/opt/trn_rl_repo")

import numpy as np
import ml_dtypes

import concourse.bass as bass
import concourse.mybir as mybir
import concourse.tile as tile
from concourse import bacc
from concourse.bass_utils import run_bass_kernel_spmd

BF16 = ml_dtypes.bfloat16
F32 = mybir.dt.float32
DT = mybir.dt.bfloat16

B, T, C = 4, 2048, 2048
H, HK, D = 16, 4, 128
P = 128
NCORES = 8
G = 2            # tensor-parallel groups
HL = H // G      # q heads per core (8)
HKL = HK // G    # kv heads per core (2)
DL = HL * D      # local q width (1024)
DKL = HKL * D    # local kv width (256)
KC = C // P      # 16 contraction chunks
NT = T // P      # 16 token tiles of 128
NJ = T // 512    # 4 q column tiles of 512
SCALE = 1.0 / float(np.sqrt(D))

_NC_CACHE = None


def _build_nc():
    nc = bacc.Bacc()

    xT = nc.declare_dram_parameter("xT", [C, T], DT, isOutput=False)
    wq = nc.declare_dram_parameter("wq", [C, DL], DT, isOutput=False)
    wk = nc.declare_dram_parameter("wk", [C, DKL], DT, isOutput=False)
    wv = nc.declare_dram_parameter("wv", [C, DKL], DT, isOutput=False)
    wc = nc.declare_dram_parameter("wc", [DL, C], DT, isOutput=False)
    cosb = nc.declare_dram_parameter("cosb", [P, T], DT, isOutput=False)
    sinb = nc.declare_dram_parameter("sinb", [P, T], DT, isOutput=False)
    unperm = nc.declare_dram_parameter("unperm", [P, P], DT, isOutput=False)
    tri = nc.declare_dram_parameter("tri", [P, P], DT, isOutput=False)
    ident = nc.declare_dram_parameter("ident", [P, P], DT, isOutput=False)

    y_out = nc.declare_dram_parameter("y", [T, C], F32, isOutput=True)
    k_out = nc.declare_dram_parameter("ko", [T, DKL], DT, isOutput=True)
    v_out = nc.declare_dram_parameter("vo", [T, DKL], DT, isOutput=True)

    with tile.TileContext(nc) as tc:
        with (
            tc.tile_pool(name="const", bufs=1) as cpool,
            tc.tile_pool(name="persist", bufs=1) as persist,
        ):
            cos_sb = cpool.tile([P, T], DT, tag="cos")
            sin_sb = cpool.tile([P, T], DT, tag="sin")
            up_sb = cpool.tile([P, P], DT, tag="unperm")
            tri_sb = cpool.tile([P, P], DT, tag="tri")
            id_sb = cpool.tile([P, P], DT, tag="ident")
            ones_sb = cpool.tile([P, P], DT, tag="ones")
            nc.sync.dma_start(cos_sb[:], cosb[:])
            nc.sync.dma_start(sin_sb[:], sinb[:])
            nc.sync.dma_start(up_sb[:], unperm[:])
            nc.sync.dma_start(tri_sb[:], tri[:])
            nc.sync.dma_start(id_sb[:], ident[:])
            nc.vector.memset(ones_sb[:], 1.0)

            qT_sb = persist.tile([P, HL, T], DT, tag="qT")
            kT_sb = persist.tile([P, HKL, T], DT, tag="kT")
            v_sb = persist.tile([P, NT, DKL], DT, tag="v")

            # ---------------- Phase A: projections + RoPE -------------
            with tc.tile_pool(name="proj", bufs=1) as proj:
                xT_sb = proj.tile([P, KC, T], DT, tag="xT")
                wq_sb = proj.tile([P, KC, DL], DT, tag="wq")
                wk_sb = proj.tile([P, KC, DKL], DT, tag="wk")
                wv_sb = proj.tile([P, KC, DKL], DT, tag="wv")
                # Split the loads per chunk so matmuls start after chunk 0.
                for kc in range(KC):
                    nc.sync.dma_start(wq_sb[:, kc], wq[kc * P:(kc + 1) * P, :])
                    nc.sync.dma_start(xT_sb[:, kc], xT[kc * P:(kc + 1) * P, :])
                    nc.sync.dma_start(wk_sb[:, kc], wk[kc * P:(kc + 1) * P, :])
                    nc.sync.dma_start(wv_sb[:, kc], wv[kc * P:(kc + 1) * P, :])

                with tc.tile_pool(name="psA", bufs=2, space="PSUM") as psA, \
                        nc.named_scope("projqk"):
                    # k and q projections (both get RoPE); k first so
                    # attention head 0 unblocks as early as possible.
                    targets = [("k", kh) for kh in range(HKL)] + [
                        ("q", h) for h in range(HL)
                    ]
                    for kind, idx in targets:
                        ps = [
                            psA.tile([P, 512], F32, tag=f"proj{j}", name=f"proj{j}")
                            for j in range(NJ)
                        ]
                        for kc in range(KC):
                            if kind == "q":
                                lhsT = wq_sb[:, kc, idx * D:(idx + 1) * D]
                            else:
                                lhsT = wk_sb[:, kc, idx * D:(idx + 1) * D]
                            for j in range(NJ):
                                nc.tensor.matmul(
                                    ps[j][:],
                                    lhsT,
                                    xT_sb[:, kc, j * 512:(j + 1) * 512],
                                    start=(kc == 0),
                                    stop=(kc == KC - 1),
                                )
                        stage = proj.tile([P, T], DT, tag="ropestage", bufs=2)
                        for j in range(NJ):
                            nc.scalar.copy(stage[:, j * 512:(j + 1) * 512], ps[j][:])
                        rot = proj.tile([P, T], DT, tag="rot", bufs=2)
                        nc.sync.dma_start(rot[0:64, :], stage[64:128, :])
                        nc.sync.dma_start(rot[64:128, :], stage[0:64, :])
                        dest = (
                            qT_sb[:, idx, :] if kind == "q" else kT_sb[:, idx, :]
                        )
                        nc.vector.tensor_mul(dest, stage[:], cos_sb[:])
                        tmp = proj.tile([P, T], DT, tag="ropetmp", bufs=2)
                        nc.vector.tensor_mul(tmp[:], rot[:], sin_sb[:])
                        nc.vector.tensor_add(dest, dest, tmp[:])

                with tc.tile_pool(name="psA2", bufs=2, space="PSUM") as psA2, \
                        nc.named_scope("vproj"):
                    # v projection: v[t, d] tiles (needs t on partitions)
                    for tt in range(NT):
                        vp = psA2.tile([P, DKL], F32, tag="vp", bufs=3)
                        for kc in range(KC):
                            nc.tensor.matmul(
                                vp[:],
                                xT_sb[:, kc, tt * P:(tt + 1) * P],
                                wv_sb[:, kc, :],
                                start=(kc == 0),
                                stop=(kc == KC - 1),
                            )
                        nc.scalar.copy(v_sb[:, tt, :], vp[:])
                    nc.sync.dma_start(
                        v_out.rearrange("(o p) d -> p o d", p=P), v_sb[:]
                    )

                    # k output: un-permute head dim back to interleaved
                    # layout via matmul with permutation matrix.
                    for kh in range(HKL):
                        kst = proj.tile([P, NT, P], DT, tag="kout", bufs=2)
                        for tt in range(NT):
                            kp = psA2.tile([P, P], F32, tag="ktr", bufs=3)
                            nc.tensor.matmul(
                                kp[:],
                                kT_sb[:, kh, tt * P:(tt + 1) * P],
                                up_sb[:],
                                start=True,
                                stop=True,
                            )
                            nc.scalar.copy(kst[:, tt, :], kp[:])
                        nc.sync.dma_start(
                            k_out.rearrange("(o p) d -> p o d", p=P)[
                                :, :, kh * D:(kh + 1) * D
                            ],
                            kst[:],
                        )

            # ---------------- Phases B+C: attention + out-proj --------
            with tc.tile_pool(name="post", bufs=1) as post:
                yT_sb = post.tile([P, HL, T], DT, tag="yT")
                # Hoist the Wc load so it overlaps the attention phase.
                wc_sb = post.tile([P, HL, C], DT, tag="wc")
                for kc in range(HL):
                    nc.sync.dma_start(wc_sb[:, kc], wc[kc * P:(kc + 1) * P, :])

                with (
                    tc.tile_pool(name="attn", bufs=1) as attn,
                    tc.tile_pool(name="psB", bufs=1, space="PSUM") as psB,
                    nc.named_scope("attn"),
                ):
                    for hp in range(HL // 2):
                      for j in range(NJ):
                        for h in (2 * hp, 2 * hp + 1):
                            kh = h // (HL // HKL)
                            yps = psB.tile([P, 512], F32, tag="yT", bufs=3)
                            dps = psB.tile([P, 512], F32, tag="den", bufs=3)
                            nkt = 4 * j + 4
                            for kt in range(nkt):
                                col0 = max(0, kt * P - j * 512)
                                n = 512 - col0
                                sps = psB.tile([P, 512], F32, tag="s", bufs=2)
                                diag = kt >= 4 * j
                                if diag:
                                    # additive causal mask (-1e5 above the
                                    # diagonal) folded into the score psum:
                                    # exp then gives exact zeros on PE alone.
                                    nc.tensor.matmul(
                                        sps[:, 0:P],
                                        id_sb[:],
                                        tri_sb[:],
                                        start=True,
                                        stop=False,
                                        skip_group_check=True,
                                    )
                                nc.tensor.matmul(
                                    sps[:, :n],
                                    kT_sb[:, kh, kt * P:(kt + 1) * P],
                                    qT_sb[:, h, j * 512 + col0:(j + 1) * 512],
                                    start=not diag,
                                    stop=True,
                                    skip_group_check=True,
                                )
                                ex = attn.tile([P, 512], DT, tag="exp", bufs=6)
                                nc.scalar.activation(
                                    ex[:, :n],
                                    sps[:, :n],
                                    mybir.ActivationFunctionType.Exp,
                                    scale=SCALE,
                                )
                                nc.tensor.matmul(
                                    yps[:, col0:],
                                    v_sb[:, kt, kh * D:(kh + 1) * D],
                                    ex[:, :n],
                                    start=(kt == 0),
                                    stop=(kt == nkt - 1),
                                    skip_group_check=True,
                                )
                                # denominator, broadcast over all partitions
                                # by an all-ones stationary
                                nc.tensor.matmul(
                                    dps[:, col0:],
                                    ones_sb[:],
                                    ex[:, :n],
                                    start=(kt == 0),
                                    stop=(kt == nkt - 1),
                                    skip_group_check=True,
                                )
                            rec = attn.tile([P, 512], F32, tag="rec", bufs=3)
                            nc.vector.reciprocal(rec[:], dps[:])
                            nc.vector.tensor_mul(
                                yT_sb[:, h, j * 512:(j + 1) * 512],
                                yps[:],
                                rec[:],
                            )

                # out projection: y[t, :] = sum_h yT_h.T @ Wc rows
                with (
                    tc.tile_pool(name="oproj", bufs=1) as op,
                    tc.tile_pool(name="psC", bufs=2, space="PSUM") as psC,
                    nc.named_scope("oproj"),
                ):
                    for tt in range(NT):
                        ost = op.tile([P, C], F32, tag="ost", bufs=3)
                        for ncol in range(NJ):
                            ops_ = psC.tile([P, 512], F32, tag="o", bufs=4)
                            for hc in range(HL):
                                nc.tensor.matmul(
                                    ops_[:],
                                    yT_sb[:, hc, tt * P:(tt + 1) * P],
                                    wc_sb[:, hc, ncol * 512:(ncol + 1) * 512],
                                    start=(hc == 0),
                                    stop=(hc == HL - 1),
                                )
                            nc.vector.tensor_copy(
                                ost[:, ncol * 512:(ncol + 1) * 512], ops_[:]
                            )
                        nc.sync.dma_start(y_out[tt * P:(tt + 1) * P, :], ost[:])

    nc.finalize()
    return nc


def _get_nc():
    global _NC_CACHE
    if _NC_CACHE is None:
        _NC_CACHE = _build_nc()
    return _NC_CACHE


def _host_prep(x, freqs_cos, freqs_sin, Wq, Wk, Wv, Wc):
    """Build the 8 per-core input maps (numpy, bf16)."""
    perm = np.empty(P, dtype=np.int64)
    perm[0:64] = np.arange(64) * 2        # real parts -> partitions 0..63
    perm[64:128] = np.arange(64) * 2 + 1  # imag parts -> partitions 64..127

    cosT = np.ascontiguousarray(freqs_cos.T)  # [64, T]
    sinT = np.ascontiguousarray(freqs_sin.T)
    cosb = np.concatenate([cosT, cosT], axis=0).astype(BF16)
    sinb = np.concatenate([-sinT, sinT], axis=0).astype(BF16)

    unperm = np.zeros((P, P), dtype=BF16)
    unperm[np.arange(P), perm] = 1.0  # M[d, perm[d]] = 1

    # additive causal mask: 0 where allowed (q >= kv), -1e5 above diagonal
    tri = np.where(
        np.arange(P)[None, :] >= np.arange(P)[:, None], 0.0, -1.0e5
    ).astype(BF16)
    ident = np.eye(P, dtype=BF16)

    Wq_p = (
        Wq.reshape(C, H, D)[:, :, perm].reshape(C, H * D).astype(BF16)
    )
    Wk_p = (
        Wk.reshape(C, HK, D)[:, :, perm].reshape(C, HK * D).astype(BF16)
    )
    Wv_b = Wv.astype(BF16)
    Wc_b = Wc.astype(BF16)

    in_maps = []
    for core in range(NCORES):
        g = core // B
        b = core % B
        xT_c = np.ascontiguousarray(x[b].T).astype(BF16)
        in_maps.append(
            {
                "xT": xT_c,
                "wq": np.ascontiguousarray(Wq_p[:, g * DL:(g + 1) * DL]),
                "wk": np.ascontiguousarray(Wk_p[:, g * DKL:(g + 1) * DKL]),
                "wv": np.ascontiguousarray(Wv_b[:, g * DKL:(g + 1) * DKL]),
                "wc": np.ascontiguousarray(Wc_b[g * DL:(g + 1) * DL, :]),
                "cosb": cosb,
                "sinb": sinb,
                "unperm": unperm,
                "tri": tri,
                "ident": ident,
            }
        )
    return in_maps


def kernel(x, freqs_cos, freqs_sin, Wq, Wk, Wv, Wc, _trace=False, _trace_kwargs=None):
    x = np.asarray(x, dtype=np.float32)
    freqs_cos = np.asarray(freqs_cos, dtype=np.float32)
    freqs_sin = np.asarray(freqs_sin, dtype=np.float32)
    Wq = np.asarray(Wq, dtype=np.float32)
    Wk = np.asarray(Wk, dtype=np.float32)
    Wv = np.asarray(Wv, dtype=np.float32)
    Wc = np.asarray(Wc, dtype=np.float32)

    in_maps = _host_prep(x, freqs_cos, freqs_sin, Wq, Wk, Wv, Wc)
    nc = _get_nc()
    res = run_bass_kernel_spmd(
        nc, in_maps, core_ids=list(range(NCORES)), trace=_trace,
        **(_trace_kwargs or {}),
    )
    results = res.results

    y = np.empty((B, T, C), dtype=np.float32)
    present_k = np.empty((B, T, HK, D), dtype=np.float32)
    present_v = np.empty((B, T, HK, D), dtype=np.float32)
    for b in range(B):
        y[b] = results[b]["y"] + results[B + b]["y"]
        for g in range(G):
            rb = results[g * B + b]
            ko = np.asarray(rb["ko"], dtype=np.float32)
            vo = np.asarray(rb["vo"], dtype=np.float32)
            for kh in range(HKL):
                present_k[b, :, g * HKL + kh, :] = ko[:, kh * D:(kh + 1) * D]
                present_v[b, :, g * HKL + kh, :] = vo[:, kh * D:(kh + 1) * D]

    kernel.last_exec_time_ns = res.exec_time_ns
    kernel.last_results = res
    return (y, present_k, present_v)
